# revision 1
# baseline (speedup 1.0000x reference)
"""Trainium2 Bass kernel for nn_NodeModel (GNN message passing).

  out = relu(concat([x, scatter_mean(edge_attr, col), u[batch]]) @ W1 + b1) @ W2 + b2

Strategy (8 NeuronCores, data-parallel over destination nodes):
  * Host: sort edges by destination node (col). Every node has degree <= 64
    (data max is 58), so each node's edges are padded to exactly DEG=64
    "edge slots"; edge values are pre-scaled by 1/count so the sum over
    slots directly yields scatter_mean. Nodes are partitioned contiguously
    across the 8 cores (12500 nodes/core -> 100 windows of 128 node slots).
  * Device, per core: a GPSIMD accumulate-DMA streams the DEG edge-slot
    planes from HBM and sums them into an SBUF tile gsn[128 nodes, 16]
    per window (the segment reduction happens inside the DMA engines).
    A PE transpose turns gsn into e_aggT[16, 128], then the MLP runs with
    nodes on the free dim: psH = W1e.T@e_aggT + W1xu.T@xuT (PSUM),
    relu+bias on ACT, psO = W2.T@hid, bias on ACT, DMA out.
  * No cross-core communication: edges live with their destination node.
"""

import numpy as np

try:
    import ml_dtypes

    _BF16 = np.dtype(ml_dtypes.bfloat16)
except Exception:  # pragma: no cover
    _BF16 = None

F_E, F_X, F_U, H, F_OUT = 16, 64, 64, 128, 64
XU = F_X + F_U  # 128

CFG = dict(
    n_cores=8,
    npc=12500,   # real nodes per core
    wpc=100,     # windows (128 node slots) per core
    chw=20,      # windows per edge-stream chunk
    b=4,         # windows per MLP batch group
    deg=64,      # padded degree (edge slots per node)
    use_accum_dma=False,
    pool_split=False,  # pre-add edge-slot halves on GpSimd before DVE reduce
    edge_dt="bf16",
    xu_dt="bf16",
    w_dt="bf16",
)

_CACHE = {}


def _npdt(name):
    return _BF16 if name == "bf16" else np.dtype(np.float32)


def _mydt(name, mybir):
    return mybir.dt.bfloat16 if name == "bf16" else mybir.dt.float32


# ---------------------------------------------------------------- host side
def _preprocess(inputs, cfg):
    NC, NPC, WPC, CHW, DEG = (
        cfg["n_cores"], cfg["npc"], cfg["wpc"], cfg["chw"], cfg["deg"],
    )
    NCH = WPC // CHW
    SLOTS = WPC * 128
    edt = _npdt(cfg["edge_dt"])
    xdt = _npdt(cfg["xu_dt"])
    wdt = _npdt(cfg["w_dt"])

    x = np.asarray(inputs["x"], np.float32)
    ea = np.asarray(inputs["edge_attr"], np.float32)
    u = np.asarray(inputs["u"], np.float32)
    W1 = np.asarray(inputs["W1"], np.float32)
    b1 = np.asarray(inputs["b1"], np.float32)
    W2 = np.asarray(inputs["W2"], np.float32)
    b2 = np.asarray(inputs["b2"], np.float32)
    col = np.asarray(np.asarray(inputs["edge_index"])[1], np.int64)
    batch = np.asarray(inputs["batch"], np.int64)

    N, E = x.shape[0], col.shape[0]
    assert N == NC * NPC, (N, NC, NPC)

    cnt = np.bincount(col, minlength=N)
    assert cnt.max() <= DEG, f"max degree {cnt.max()} > DEG {DEG}"
    invc = np.zeros(N, np.float32)
    nz = cnt > 0
    invc[nz] = 1.0 / cnt[nz]

    order = np.argsort(col, kind="stable")
    cols = col[order]
    eas = ea[order] * invc[cols][:, None]  # pre-scaled by 1/count

    starts = np.concatenate([[0], np.cumsum(cnt)[:-1]])
    rank = np.arange(E, dtype=np.int64) - starts[cols]  # slot within node
    c = cols // NPC
    m = cols - c * NPC
    w = m >> 7          # window within core
    p = m & 127         # node slot within window
    ch = w // CHW
    wi = w - ch * CHW

    if cfg["use_accum_dma"]:
        # layout [core][chunk][slot e][p][wi*16+f]
        A = np.zeros((NC, NCH, DEG, 128, CHW * F_E), edt)
        rows = (((c * NCH + ch) * DEG + rank) * 128 + p) * CHW + wi
        A.reshape(-1, F_E)[rows] = eas.astype(edt)
    else:
        # layout [core][w][p][f][e]
        tmp = np.zeros((NC, WPC, 128, DEG, F_E), edt)
        rows = ((c * WPC + w) * 128 + p) * DEG + rank
        tmp.reshape(-1, F_E)[rows] = eas.astype(edt)
        A = np.ascontiguousarray(tmp.swapaxes(3, 4))  # [NC, WPC, 128, 16, DEG]

    # node features: concat(x, u[batch]) transposed, padded to SLOTS
    xu = np.concatenate([x, u[batch]], axis=1)  # [N, 128]
    xuT = np.zeros((NC, XU, SLOTS), xdt)
    xr = xu.reshape(NC, NPC, XU)
    for ci in range(NC):
        xuT[ci, :, :NPC] = xr[ci].T.astype(xdt)

    W1xu = np.ascontiguousarray(
        np.concatenate([W1[0:F_X], W1[F_X + F_E:]], axis=0), dtype=wdt
    )  # [128, 128]
    W1e = np.ascontiguousarray(W1[F_X:F_X + F_E], dtype=wdt)  # [16, 128]
    W2c = np.ascontiguousarray(W2, dtype=wdt)  # [128, 64]
    ident = np.eye(128, dtype=np.float32)

    common = dict(
        w1xu=W1xu, w1e=W1e, w2=W2c,
        b1=np.ascontiguousarray(b1.reshape(H, 1), np.float32),
        b2=np.ascontiguousarray(b2.reshape(F_OUT, 1), np.float32),
        ident=ident,
    )
    in_maps = []
    for ci in range(NC):
        im = dict(common)
        im["edges"] = A[ci]
        im["xut"] = xuT[ci]
        in_maps.append(im)
    return in_maps


def _postprocess(results, cfg):
    NC, NPC, WPC, B = cfg["n_cores"], cfg["npc"], cfg["wpc"], cfg["b"]
    SLOTS = WPC * 128
    out = np.empty((NC * NPC, F_OUT), np.float32)
    for ci in range(NC):
        o = np.asarray(results[ci]["outT"])  # [NB, 64, B*128]
        o = o.transpose(1, 0, 2).reshape(F_OUT, SLOTS)
        out[ci * NPC:(ci + 1) * NPC] = o[:, :NPC].T
    return out


# ------------------------------------------------------------- device side
def _build(cfg):
    import concourse.bacc as bacc
    import concourse.bass as bass
    import concourse.mybir as mybir
    import concourse.tile as tile
    from contextlib import ExitStack

    NC, WPC, CHW, B, DEG = (
        cfg["n_cores"], cfg["wpc"], cfg["chw"], cfg["b"], cfg["deg"],
    )
    NCH = WPC // CHW
    NB = WPC // B
    GPB = CHW // B  # B-groups per chunk
    SLOTS = WPC * 128
    f32 = mybir.dt.float32
    edt = _mydt(cfg["edge_dt"], mybir)
    xdt = _mydt(cfg["xu_dt"], mybir)
    wdt = _mydt(cfg["w_dt"], mybir)
    AF = mybir.ActivationFunctionType

    nc = bacc.Bacc("TRN2", target_bir_lowering=False)

    if cfg["use_accum_dma"]:
        edges_d = nc.dram_tensor(
            "edges", [NCH, DEG, 128, CHW * F_E], edt, kind="ExternalInput")
    else:
        edges_d = nc.dram_tensor(
            "edges", [WPC, 128, F_E, DEG], edt, kind="ExternalInput")
    xut_d = nc.dram_tensor("xut", [XU, SLOTS], xdt, kind="ExternalInput")
    w1xu_d = nc.dram_tensor("w1xu", [XU, H], wdt, kind="ExternalInput")
    w1e_d = nc.dram_tensor("w1e", [F_E, H], wdt, kind="ExternalInput")
    w2_d = nc.dram_tensor("w2", [H, F_OUT], wdt, kind="ExternalInput")
    b1_d = nc.dram_tensor("b1", [H, 1], f32, kind="ExternalInput")
    b2_d = nc.dram_tensor("b2", [F_OUT, 1], f32, kind="ExternalInput")
    ident_d = nc.dram_tensor("ident", [128, 128], f32, kind="ExternalInput")
    out_d = nc.dram_tensor("outT", [NB, F_OUT, B * 128], f32,
                           kind="ExternalOutput")

    with tile.TileContext(nc) as tc, ExitStack() as ctx:
        consts = ctx.enter_context(tc.tile_pool(name="consts", bufs=1))
        gsn_pool = ctx.enter_context(tc.tile_pool(name="gsn", bufs=2))
        ea_pool = ctx.enter_context(tc.tile_pool(name="ea", bufs=2))
        hid_pool = ctx.enter_context(tc.tile_pool(name="hid", bufs=2))
        out_pool = ctx.enter_context(tc.tile_pool(name="outs", bufs=3))
        pse_pool = ctx.enter_context(
            tc.tile_pool(name="pse", bufs=2, space="PSUM"))
        psh_pool = ctx.enter_context(
            tc.tile_pool(name="psh", bufs=2, space="PSUM"))
        pso_pool = ctx.enter_context(
            tc.tile_pool(name="pso", bufs=2, space="PSUM"))
        if not cfg["use_accum_dma"]:
            edge_pool = ctx.enter_context(tc.tile_pool(name="edges", bufs=4))
            if cfg.get("pool_split"):
                tmp_pool = ctx.enter_context(tc.tile_pool(name="tmph", bufs=3))

        ident_t = consts.tile([128, 128], f32)
        nc.sync.dma_start(ident_t[:], ident_d[:])
        w1xu_t = consts.tile([XU, H], wdt)
        nc.sync.dma_start(w1xu_t[:], w1xu_d[:])
        w1e_t = consts.tile([F_E, H], wdt)
        nc.sync.dma_start(w1e_t[:], w1e_d[:])
        w2_t = consts.tile([H, F_OUT], wdt)
        nc.sync.dma_start(w2_t[:], w2_d[:])
        b1_t = consts.tile([H, 1], f32)
        nc.sync.dma_start(b1_t[:], b1_d[:])
        b2_t = consts.tile([F_OUT, 1], f32)
        nc.sync.dma_start(b2_t[:], b2_d[:])
        xut_t = consts.tile([XU, SLOTS], xdt)
        nc.sync.dma_start(xut_t[:], xut_d[:])

        for chi in range(NCH):
            gsn = gsn_pool.tile([128, CHW * F_E], f32)
            if cfg["use_accum_dma"]:
                nc.gpsimd.memset(gsn[:], 0.0)
                src = edges_d[chi].rearrange("e p f -> p e f")
                dst_ap = gsn[:]
                dst = bass.AP(
                    dst_ap.tensor, dst_ap.offset,
                    [dst_ap.ap[0], [0, DEG]] + dst_ap.ap[1:],
                )
                nc.gpsimd.dma_start(dst, src, accum_op=mybir.AluOpType.add)
            else:
                for wi in range(CHW):
                    wg = chi * CHW + wi
                    et = edge_pool.tile([128, F_E * DEG], edt)
                    nc.sync.dma_start(
                        et[:], edges_d[wg].rearrange("p f e -> p (f e)"))
                    ev = et[:].rearrange("p (f e) -> p f e", e=DEG)
                    if cfg.get("pool_split"):
                        hd = DEG // 2
                        tmp = tmp_pool.tile([128, F_E * hd], edt)
                        tv = tmp[:].rearrange("p (f e) -> p f e", e=hd)
                        nc.gpsimd.scalar_tensor_tensor(
                            out=tv, in0=ev[:, :, 0:hd], scalar=1.0,
                            in1=ev[:, :, hd:DEG],
                            op0=mybir.AluOpType.mult,
                            op1=mybir.AluOpType.add,
                        )
                        ev = tv
                    nc.vector.tensor_reduce(
                        out=gsn[:, wi * F_E:(wi + 1) * F_E],
                        in_=ev,
                        axis=mybir.AxisListType.X,
                        op=mybir.AluOpType.add,
                    )

            for bi in range(GPB):
                g = chi * GPB + bi
                pse = pse_pool.tile([F_E, B * 128], f32)
                for j in range(B):
                    wi = bi * B + j
                    nc.tensor.transpose(
                        pse[:, j * 128:(j + 1) * 128],
                        gsn[:, wi * F_E:(wi + 1) * F_E],
                        ident_t[:],
                    )
                ea = ea_pool.tile([F_E, B * 128], wdt)
                nc.vector.tensor_copy(ea[:], pse[:])

                psh = psh_pool.tile([H, B * 128], f32)
                for j in range(B):
                    wg = g * B + j
                    nc.tensor.matmul(
                        psh[:, j * 128:(j + 1) * 128],
                        w1e_t[:], ea[:, j * 128:(j + 1) * 128],
                        start=True, stop=False,
                    )
                    nc.tensor.matmul(
                        psh[:, j * 128:(j + 1) * 128],
                        w1xu_t[:], xut_t[:, wg * 128:(wg + 1) * 128],
                        start=False, stop=True,
                    )
                hid = hid_pool.tile([H, B * 128], wdt)
                nc.scalar.activation(hid[:], psh[:], AF.Relu,
                                     bias=b1_t[:], scale=1.0)

                pso = pso_pool.tile([F_OUT, B * 128], f32)
                for j in range(B):
                    nc.tensor.matmul(
                        pso[:, j * 128:(j + 1) * 128],
                        w2_t[:], hid[:, j * 128:(j + 1) * 128],
                        start=True, stop=True,
                    )
                outs = out_pool.tile([F_OUT, B * 128], f32)
                nc.scalar.activation(outs[:], pso[:], AF.Identity,
                                     bias=b2_t[:], scale=1.0)
                nc.sync.dma_start(out_d[g], outs[:])

    nc.finalize()
    return nc


def _get_program(cfg):
    key = tuple(sorted((k, v) for k, v in cfg.items()))
    if key not in _CACHE:
        _CACHE[key] = _build(cfg)
    return _CACHE[key]


def run(inputs, cfg=None, trace=False):
    from concourse.bass_utils import run_bass_kernel_spmd

    cfg = dict(CFG if cfg is None else cfg)
    nc = _get_program(cfg)
    in_maps = _preprocess(inputs, cfg)
    res = run_bass_kernel_spmd(
        nc, in_maps, list(range(cfg["n_cores"])), trace=trace)
    out = _postprocess(res.results, cfg)
    return out, res


def kernel(**inputs):
    return run(inputs)[0]



# revision 10
# speedup vs baseline: 1.9520x; 1.9520x over previous
"""Trainium2 Bass kernel for nn_NodeModel (GNN message passing).

  out = relu(concat([x, scatter_mean(edge_attr, col), u[batch]]) @ W1 + b1) @ W2 + b2

Strategy (8 NeuronCores, data-parallel over destination nodes):
  * Nodes are partitioned contiguously across the 8 cores (12500/core);
    edges live with their destination node, so scatter_mean is a purely
    local segment reduction (no cross-core traffic).
  * Within a core, nodes are permuted in degree-descending order and
    grouped into 100 windows of 128 node slots. Each window w is padded
    to cap[w] = max degree in that window (rounded up to a multiple of
    2, shared across cores) -- ~3% padding instead of the 2x a global
    max-degree pad costs. Edge values ship as fp8 (e3m4); the 1/count
    scaling of scatter_mean is applied on device as a per-partition
    activation scale, so quantization happens at the natural ~N(0,1)
    scale of edge_attr.
  * u[batch] is never materialized on the wire: host precomputes
    W1u_eff = u @ W1[80:144] (exact, f32) and ships a 0/1 one-hot
    graph-membership matrix in fp8 (exact). Its contribution enters the
    hidden-layer PSUM as one extra matmul W1u_eff.T @ onehot.
  * Device, per core and per window: DMA the fp8 edge block
    [128, 16*cap], DVE-reduce over the cap axis, scale by 1/count,
    PE-transpose to [16, 128]. Per group of 4 windows: psum
    [128H, 512] = W1e.T@eT + W1x.T@xT + W1u_eff.T@onehot, ReLU+bias,
    [64, 512] = W2.T@hid, +bias, DMA out in f16.
"""

import numpy as np
import ml_dtypes

_BF16 = np.dtype(ml_dtypes.bfloat16)
_FP8E3 = np.dtype(ml_dtypes.float8_e3m4)
_F16 = np.dtype(np.float16)

F_E, F_X, F_U, H, F_OUT = 16, 64, 64, 128, 64
N_NODES, N_GRAPHS = 100000, 64
NC, NPC, WPC, B = 8, 12500, 100, 4
SLOTS = WPC * 128          # 12800 node slots per core
NB = WPC // B              # MLP groups per core

XT_FP8 = True  # ship x in fp8e3 (6.4MB) instead of bf16 (12.8MB)

_PROGRAM_CACHE = {}
_RUNNER_CACHE = {}


# ---------------------------------------------------------------- host side
def _plan_and_preprocess(inputs):
    x = np.asarray(inputs["x"], np.float32)
    ea = np.asarray(inputs["edge_attr"], np.float32)
    u = np.asarray(inputs["u"], np.float32)
    W1 = np.asarray(inputs["W1"], np.float32)
    b1 = np.asarray(inputs["b1"], np.float32)
    W2 = np.asarray(inputs["W2"], np.float32)
    b2 = np.asarray(inputs["b2"], np.float32)
    col = np.asarray(np.asarray(inputs["edge_index"])[1], np.int64)
    batch = np.asarray(inputs["batch"], np.int64)

    N, E = x.shape[0], col.shape[0]
    assert N == NC * NPC, (N, NC, NPC)

    cnt = np.bincount(col, minlength=N)
    invc = (1.0 / np.maximum(cnt, 1)).astype(np.float32)

    # per-core degree-descending node permutation; shared window caps
    cnt2 = cnt.reshape(NC, NPC)
    order = np.argsort(-cnt2, axis=1, kind="stable")          # [NC, NPC]
    slot_of_local = np.empty((NC, NPC), np.int64)
    np.put_along_axis(slot_of_local, order,
                      np.broadcast_to(np.arange(NPC), (NC, NPC)), axis=1)
    deg_sorted = np.take_along_axis(cnt2, order, axis=1)
    padded = np.zeros((NC, SLOTS), np.int64)
    padded[:, :NPC] = deg_sorted
    caps = padded.reshape(NC, WPC, 128).max(axis=2).max(axis=0)
    caps = np.maximum(caps, 2)
    caps = ((caps + 1) // 2 * 2).astype(np.int64)             # [WPC]

    offs = np.zeros(WPC, np.int64)
    offs[1:] = np.cumsum(caps[:-1]) * (128 * F_E)
    total = int(caps.sum()) * 128 * F_E                        # elems per core

    # edge scatter into per-core flat fp8 arrays (window blocks [128, 16, cap])
    order_e = np.argsort(col, kind="stable")
    cols = col[order_e]
    eas8 = ea[order_e].astype(_FP8E3)
    starts = np.concatenate([[0], np.cumsum(cnt)[:-1]])
    rank = np.arange(E, dtype=np.int64) - starts[cols]
    c_of = cols // NPC
    s_of = slot_of_local[c_of, cols - c_of * NPC]
    w_of = s_of >> 7
    p_of = s_of & 127
    capw = caps[w_of]
    base = (c_of * total + offs[w_of] + p_of * (F_E * capw) + rank).astype(np.int32)
    cap32 = capw.astype(np.int32)
    A = np.zeros(NC * total, _FP8E3)
    for f in range(F_E):
        A[base + np.int32(f) * cap32] = eas8[:, f]
    A = A.reshape(NC, total)

    # node features transposed into slot order
    rows = np.arange(NC)[:, None]
    xdt = _FP8E3 if XT_FP8 else _BF16
    xp = np.zeros((NC, SLOTS, F_X), xdt)
    xp[rows, slot_of_local] = x.reshape(NC, NPC, F_X).astype(xdt)
    xt = np.ascontiguousarray(xp.transpose(0, 2, 1))           # [NC, 64, SLOTS]

    oh = np.zeros((NC, N_GRAPHS, SLOTS), _FP8E3)
    oh[rows, batch.reshape(NC, NPC), slot_of_local] = 1.0

    iv = np.ones((NC, SLOTS), np.float32)
    iv[rows, slot_of_local] = invc.reshape(NC, NPC)
    ivt = np.ascontiguousarray(
        iv.reshape(NC, WPC, 128).transpose(0, 2, 1))           # [NC, 128, WPC]

    w1x = np.ascontiguousarray(W1[0:F_X], dtype=_BF16)                 # [64,128]
    w1e = np.ascontiguousarray(W1[F_X:F_X + F_E], dtype=_BF16)         # [16,128]
    w1u = np.ascontiguousarray(u @ W1[F_X + F_E:], dtype=_BF16)        # [64,128]
    w2 = np.ascontiguousarray(W2, dtype=_BF16)                         # [128,64]

    common = dict(
        w1x=w1x, w1e=w1e, w1u=w1u, w2=w2,
        b1=np.ascontiguousarray(b1.reshape(H, 1), np.float32),
        b2=np.ascontiguousarray(b2.reshape(F_OUT, 1), np.float32),
        ident=np.eye(128, dtype=np.float32),
    )
    in_maps = []
    for ci in range(NC):
        im = dict(common)
        im["edges"] = A[ci]
        im["xt"] = xt[ci]
        im["oh"] = oh[ci]
        im["invc"] = ivt[ci]
        in_maps.append(im)

    plan = dict(caps=tuple(int(c) for c in caps),
                offs=offs, total=total, slot_of_local=slot_of_local)
    return plan, in_maps


def _postprocess(results, plan):
    slot_of_local = plan["slot_of_local"]
    out = np.empty((NC * NPC, F_OUT), np.float32)
    for ci in range(NC):
        o = np.asarray(results[ci]["outT"])                    # [NB, 64, 512] f16
        o2 = o.transpose(1, 0, 2).reshape(F_OUT, SLOTS)
        out[ci * NPC:(ci + 1) * NPC] = o2[:, slot_of_local[ci]].T
    return out


# ------------------------------------------------------------- device side
def _build(caps):
    import concourse.bacc as bacc
    import concourse.mybir as mybir
    import concourse.tile as tile
    from contextlib import ExitStack

    f32 = mybir.dt.float32
    bf16 = mybir.dt.bfloat16
    f16 = mybir.dt.float16
    fp8 = mybir.dt.float8e3
    AF = mybir.ActivationFunctionType

    caps = list(caps)
    offs = [0] * WPC
    for w in range(1, WPC):
        offs[w] = offs[w - 1] + caps[w - 1] * 128 * F_E
    total = offs[-1] + caps[-1] * 128 * F_E

    nc = bacc.Bacc("TRN2", target_bir_lowering=False)

    edges_d = nc.dram_tensor("edges", [total], fp8, kind="ExternalInput")
    xt_d = nc.dram_tensor("xt", [F_X, SLOTS], fp8 if XT_FP8 else bf16,
                          kind="ExternalInput")
    oh_d = nc.dram_tensor("oh", [N_GRAPHS, SLOTS], fp8, kind="ExternalInput")
    invc_d = nc.dram_tensor("invc", [128, WPC], f32, kind="ExternalInput")
    w1x_d = nc.dram_tensor("w1x", [F_X, H], bf16, kind="ExternalInput")
    w1e_d = nc.dram_tensor("w1e", [F_E, H], bf16, kind="ExternalInput")
    w1u_d = nc.dram_tensor("w1u", [N_GRAPHS, H], bf16, kind="ExternalInput")
    w2_d = nc.dram_tensor("w2", [H, F_OUT], bf16, kind="ExternalInput")
    b1_d = nc.dram_tensor("b1", [H, 1], f32, kind="ExternalInput")
    b2_d = nc.dram_tensor("b2", [F_OUT, 1], f32, kind="ExternalInput")
    ident_d = nc.dram_tensor("ident", [128, 128], f32, kind="ExternalInput")
    out_d = nc.dram_tensor("outT", [NB, F_OUT, B * 128], f16,
                           kind="ExternalOutput")

    with tile.TileContext(nc) as tc, ExitStack() as ctx:
        consts = ctx.enter_context(tc.tile_pool(name="consts", bufs=1))
        edge_pool = ctx.enter_context(tc.tile_pool(name="edges", bufs=4))
        gsn_pool = ctx.enter_context(tc.tile_pool(name="gsn", bufs=4))
        gsc_pool = ctx.enter_context(tc.tile_pool(name="gsc", bufs=2))
        ea_pool = ctx.enter_context(tc.tile_pool(name="ea", bufs=2))
        hid_pool = ctx.enter_context(tc.tile_pool(name="hid", bufs=2))
        out_pool = ctx.enter_context(tc.tile_pool(name="outs", bufs=3))
        pse_pool = ctx.enter_context(
            tc.tile_pool(name="pse", bufs=2, space="PSUM"))
        psh_pool = ctx.enter_context(
            tc.tile_pool(name="psh", bufs=2, space="PSUM"))
        pso_pool = ctx.enter_context(
            tc.tile_pool(name="pso", bufs=2, space="PSUM"))

        ident_t = consts.tile([128, 128], f32)
        nc.sync.dma_start(ident_t[:], ident_d[:])
        w1x_t = consts.tile([F_X, H], bf16)
        nc.sync.dma_start(w1x_t[:], w1x_d[:])
        w1e_t = consts.tile([F_E, H], bf16)
        nc.sync.dma_start(w1e_t[:], w1e_d[:])
        w1u_t = consts.tile([N_GRAPHS, H], bf16)
        nc.sync.dma_start(w1u_t[:], w1u_d[:])
        w2_t = consts.tile([H, F_OUT], bf16)
        nc.sync.dma_start(w2_t[:], w2_d[:])
        b1_t = consts.tile([H, 1], f32)
        nc.sync.dma_start(b1_t[:], b1_d[:])
        b2_t = consts.tile([F_OUT, 1], f32)
        nc.sync.dma_start(b2_t[:], b2_d[:])
        invc_t = consts.tile([128, WPC], f32)
        nc.sync.dma_start(invc_t[:], invc_d[:])
        if XT_FP8:
            xt8_t = consts.tile([F_X, SLOTS], fp8)
            nc.sync.dma_start(xt8_t[:], xt_d[:])
            xt_t = consts.tile([F_X, SLOTS], bf16)
            nc.vector.tensor_copy(xt_t[:], xt8_t[:])
        else:
            xt_t = consts.tile([F_X, SLOTS], bf16)
            nc.sync.dma_start(xt_t[:], xt_d[:])
        oh_t = consts.tile([N_GRAPHS, SLOTS], fp8)
        nc.sync.dma_start(oh_t[:], oh_d[:])
        ohb_t = consts.tile([N_GRAPHS, SLOTS], bf16)
        nc.vector.tensor_copy(ohb_t[:], oh_t[:])

        for g in range(NB):
            gsc = gsc_pool.tile([128, B * F_E], f32)
            for j in range(B):
                w = g * B + j
                cw = caps[w]
                et = edge_pool.tile([128, F_E * cw], fp8)
                nc.sync.dma_start(
                    et[:],
                    edges_d[offs[w]:offs[w] + 128 * F_E * cw].rearrange(
                        "(p q) -> p q", p=128),
                )
                gsn = gsn_pool.tile([128, F_E], f32)
                nc.vector.tensor_reduce(
                    out=gsn[:],
                    in_=et[:].rearrange("p (f e) -> p f e", e=cw),
                    axis=mybir.AxisListType.X,
                    op=mybir.AluOpType.add,
                )
                nc.scalar.activation(
                    gsc[:, j * F_E:(j + 1) * F_E], gsn[:], AF.Identity,
                    scale=invc_t[:, w:w + 1],
                )

            pse = pse_pool.tile([F_E, B * 128], f32)
            for j in range(B):
                nc.tensor.transpose(
                    pse[:, j * 128:(j + 1) * 128],
                    gsc[:, j * F_E:(j + 1) * F_E],
                    ident_t[:],
                )
            ea = ea_pool.tile([F_E, B * 128], bf16)
            nc.vector.tensor_copy(ea[:], pse[:])

            psh = psh_pool.tile([H, B * 128], f32)
            nc.tensor.matmul(psh[:], w1e_t[:], ea[:], start=True, stop=False)
            nc.tensor.matmul(psh[:], w1x_t[:],
                             xt_t[:, g * 512:(g + 1) * 512],
                             start=False, stop=False)
            nc.tensor.matmul(psh[:], w1u_t[:],
                             ohb_t[:, g * 512:(g + 1) * 512],
                             start=False, stop=True)
            hid = hid_pool.tile([H, B * 128], bf16)
            nc.scalar.activation(hid[:], psh[:], AF.Relu, bias=b1_t[:])

            pso = pso_pool.tile([F_OUT, B * 128], f32)
            nc.tensor.matmul(pso[:], w2_t[:], hid[:], start=True, stop=True)
            outs = out_pool.tile([F_OUT, B * 128], f16)
            nc.scalar.activation(outs[:], pso[:], AF.Identity, bias=b2_t[:])
            nc.sync.dma_start(out_d[g], outs[:])

    nc.finalize()
    return nc


def _get_program(caps):
    key = (caps, XT_FP8)
    if key not in _PROGRAM_CACHE:
        _PROGRAM_CACHE[key] = _build(caps)
    return _PROGRAM_CACHE[key]


# ------------------------------------------------------------- runner
class _Runner:
    """Persistent compiled executable + device-side state for one program."""

    def __init__(self, nc):
        import jax
        import jax.numpy as jnp  # noqa: F401
        from jax.sharding import Mesh, PartitionSpec, NamedSharding
        from jax.experimental.shard_map import shard_map
        import concourse.bass2jax as bass2jax
        import concourse.mybir as mybir

        self.jax = jax
        bass2jax.install_neuronx_cc_hook()

        partition_name = (nc.partition_id_tensor.name
                          if nc.partition_id_tensor else None)
        in_names, out_names, out_avals = [], [], []
        for alloc in nc.m.functions[0].allocations:
            if not isinstance(alloc, mybir.MemoryLocationSet):
                continue
            name = alloc.memorylocations[0].name
            if alloc.kind == "ExternalInput":
                if name != partition_name:
                    in_names.append(name)
            elif alloc.kind == "ExternalOutput":
                out_names.append(name)
                out_avals.append(jax.core.ShapedArray(
                    tuple(alloc.tensor_shape), mybir.dt.np(alloc.dtype)))
        n_params = len(in_names)
        in_names_all = list(in_names) + out_names
        if partition_name is not None:
            in_names_all.append(partition_name)

        def _body(*args):
            operands = list(args)
            if partition_name is not None:
                operands.append(bass2jax.partition_id_tensor())
            return tuple(bass2jax._bass_exec_p.bind(
                *operands,
                out_avals=tuple(out_avals),
                in_names=tuple(in_names_all),
                out_names=tuple(out_names),
                lowering_input_output_aliases=(),
                sim_require_finite=True,
                sim_require_nnan=True,
                nc=nc,
            ))

        devices = jax.devices()[:NC]
        mesh = Mesh(np.asarray(devices), ("core",))
        n_ops = n_params + len(out_names)
        self.sharded = jax.jit(
            shard_map(_body, mesh=mesh,
                      in_specs=(PartitionSpec("core"),) * n_ops,
                      out_specs=(PartitionSpec("core"),) * len(out_names),
                      check_rep=False),
            keep_unused=True,
        )
        self.in_names = in_names
        self.out_names = out_names
        self.out_avals = out_avals
        self.sharding = NamedSharding(mesh, PartitionSpec("core"))
        # output-named dummy operands: never read by the NEFF (our kernel
        # writes every output element), device-resident, reused every call
        self.dummy_outs = [
            jax.device_put(
                np.zeros((NC * a.shape[0], *a.shape[1:]), a.dtype),
                self.sharding)
            for a in out_avals
        ]
        self.jax.block_until_ready(self.dummy_outs)

    def concat(self, in_maps):
        return [
            np.concatenate([np.asarray(m[name]) for m in in_maps], axis=0)
            for name in self.in_names
        ]

    def put(self, concat_in):
        dev = [self.jax.device_put(a, self.sharding) for a in concat_in]
        self.jax.block_until_ready(dev)
        return dev

    def exec(self, dev_in):
        out = self.sharded(*dev_in, *self.dummy_outs)
        self.jax.block_until_ready(out)
        return out

    def exec_async(self, dev_in):
        return self.sharded(*dev_in, *self.dummy_outs)

    def fetch(self, out):
        host = [np.asarray(o) for o in out]
        return [
            {name: host[i].reshape((NC,) + self.out_avals[i].shape)[c]
             for i, name in enumerate(self.out_names)}
            for c in range(NC)
        ]


def _get_runner(caps):
    key = (caps, XT_FP8)
    if key not in _RUNNER_CACHE:
        _RUNNER_CACHE[key] = _Runner(_get_program(caps))
    return _RUNNER_CACHE[key]


def run(inputs):
    plan, in_maps = _plan_and_preprocess(inputs)
    runner = _get_runner(plan["caps"])
    dev_in = runner.put(runner.concat(in_maps))
    out = runner.exec(dev_in)
    results = runner.fetch(out)
    return _postprocess(results, plan), plan, in_maps, runner


def kernel(**inputs):
    return run(inputs)[0]


# revision 16
# speedup vs baseline: 2.7148x; 1.3908x over previous
"""Trainium2 Bass kernel for nn_NodeModel (GNN message passing).

  out = relu(concat([x, scatter_mean(edge_attr, col), u[batch]]) @ W1 + b1) @ W2 + b2

Strategy (8 NeuronCores, data-parallel over destination nodes):
  * Nodes are partitioned contiguously across the 8 cores (12500/core);
    edges live with their destination node, so scatter_mean is a purely
    local segment reduction (no cross-core traffic).
  * Within a core, nodes are permuted in degree-descending order and
    grouped into 100 windows of 128 node slots. Each window w is padded
    to cap[w] = max degree in that window (rounded up to a multiple of
    2, shared across cores) -- ~3% padding instead of the 2x a global
    max-degree pad costs. Edge values and x ship as fp8 (e3m4); the
    1/count scaling of scatter_mean is applied on device as a
    per-partition activation scale, so quantization happens at the
    natural ~N(0,1) scale of edge_attr.
  * u[batch] is never materialized: host precomputes W1u_eff =
    u @ W1[80:144] (exact, f32) and ships a 0/1 one-hot graph-membership
    matrix in fp8 (exact). Its contribution enters the hidden-layer
    PSUM as one extra matmul W1u_eff.T @ onehot.
  * Device, per core and per window: DMA the fp8 edge block
    [128, 16*cap], DVE-reduce over the cap axis, scale by 1/count,
    PE-transpose to [16, 128]. Per group of 4 windows: psum
    [128H, 512] = W1e.T@eT + W1x.T@xT + W1u_eff.T@onehot, ReLU+bias,
    [64, 512] = W2.T@hid, +bias, DMA out in f16.
  * The work is cut into 5 pipeline stages of 20 windows each; every
    stage's inputs ship as one contiguous uint8 blob per core (the
    program bitcasts slices of it). Stage puts are queued
    asynchronously and stage outputs are fetched on worker threads, so
    output D2H overlaps later input H2D (the link is full duplex).
"""

import numpy as np
import ml_dtypes
from concurrent.futures import ThreadPoolExecutor

_BF16 = np.dtype(ml_dtypes.bfloat16)
_FP8E3 = np.dtype(ml_dtypes.float8_e3m4)

F_E, F_X, F_U, H, F_OUT = 16, 64, 64, 128, 64
N_NODES, N_GRAPHS = 100000, 64
NC, NPC, WPC, B = 8, 12500, 100, 4
SLOTS = WPC * 128          # 12800 node slots per core
NB = WPC // B              # 25 MLP groups per core
STAGES = 5
WPS = WPC // STAGES        # 20 windows per stage
GPS = NB // STAGES         # 5 MLP groups per stage
SPS = WPS * 128            # 2560 slots per stage
XT_FP8 = True              # ship x in fp8e3 instead of bf16

_PROGRAM_CACHE = {}
_RUNNER_CACHE = {}


def _align(n, a=64):
    return (n + a - 1) // a * a


def _blob_layout(caps_k):
    """Byte offsets of each tensor inside a stage blob."""
    xsz = 1 if XT_FP8 else 2
    sizes = [
        ("edges", int(sum(caps_k)) * 128 * F_E),
        ("xt", F_X * SPS * xsz),
        ("oh", N_GRAPHS * SPS),
        ("invc", 128 * WPS * 4),
        ("w1x", F_X * H * 2),
        ("w1e", F_E * H * 2),
        ("w1u", N_GRAPHS * H * 2),
        ("w2", H * F_OUT * 2),
        ("b1", H * 4),
        ("b2", F_OUT * 4),
        ("ident", 128 * 128 * 4),
    ]
    offs, cur = {}, 0
    for name, sz in sizes:
        offs[name] = cur
        cur = _align(cur + sz)
    return offs, cur


# ---------------------------------------------------------------- host side
def _plan_and_preprocess(inputs):
    x = np.asarray(inputs["x"], np.float32)
    ea = np.asarray(inputs["edge_attr"], np.float32)
    u = np.asarray(inputs["u"], np.float32)
    W1 = np.asarray(inputs["W1"], np.float32)
    b1 = np.asarray(inputs["b1"], np.float32)
    W2 = np.asarray(inputs["W2"], np.float32)
    b2 = np.asarray(inputs["b2"], np.float32)
    col = np.asarray(np.asarray(inputs["edge_index"])[1], np.int64)
    batch = np.asarray(inputs["batch"], np.int64)

    N, E = x.shape[0], col.shape[0]
    assert N == NC * NPC, (N, NC, NPC)

    cnt = np.bincount(col, minlength=N)
    invc = (1.0 / np.maximum(cnt, 1)).astype(np.float32)

    # per-core degree-descending node permutation; shared window caps
    cnt2 = cnt.reshape(NC, NPC)
    order = np.argsort(-cnt2, axis=1, kind="stable")          # [NC, NPC]
    slot_of_local = np.empty((NC, NPC), np.int64)
    np.put_along_axis(slot_of_local, order,
                      np.broadcast_to(np.arange(NPC), (NC, NPC)), axis=1)
    deg_sorted = np.take_along_axis(cnt2, order, axis=1)
    padded = np.zeros((NC, SLOTS), np.int64)
    padded[:, :NPC] = deg_sorted
    caps = padded.reshape(NC, WPC, 128).max(axis=2).max(axis=0)
    caps = np.maximum(caps, 2)
    caps = ((caps + 1) // 2 * 2).astype(np.int64)             # [WPC]

    offs = np.zeros(WPC, np.int64)
    offs[1:] = np.cumsum(caps[:-1]) * (128 * F_E)
    total = int(caps.sum()) * 128 * F_E                        # bytes per core

    # edge scatter into per-core flat fp8 arrays (window blocks [128, 16, cap])
    order_e = np.argsort(col, kind="stable")
    cols = col[order_e]
    eas8 = ea[order_e].astype(_FP8E3)
    starts = np.concatenate([[0], np.cumsum(cnt)[:-1]])
    rank = np.arange(E, dtype=np.int64) - starts[cols]
    c_of = cols // NPC
    s_of = slot_of_local[c_of, cols - c_of * NPC]
    w_of = s_of >> 7
    p_of = s_of & 127
    capw = caps[w_of]
    base = (c_of * total + offs[w_of] + p_of * (F_E * capw) + rank).astype(np.int32)
    cap32 = capw.astype(np.int32)
    A = np.zeros(NC * total, _FP8E3)
    for f in range(F_E):
        A[base + np.int32(f) * cap32] = eas8[:, f]
    A = A.reshape(NC, total)

    # node features transposed into slot order
    rows = np.arange(NC)[:, None]
    xdt = _FP8E3 if XT_FP8 else _BF16
    xp = np.zeros((NC, SLOTS, F_X), xdt)
    xp[rows, slot_of_local] = x.reshape(NC, NPC, F_X).astype(xdt)
    xt = np.ascontiguousarray(xp.transpose(0, 2, 1))           # [NC, 64, SLOTS]

    oh = np.zeros((NC, N_GRAPHS, SLOTS), _FP8E3)
    oh[rows, batch.reshape(NC, NPC), slot_of_local] = 1.0

    iv = np.ones((NC, SLOTS), np.float32)
    iv[rows, slot_of_local] = invc.reshape(NC, NPC)
    ivt = np.ascontiguousarray(
        iv.reshape(NC, WPC, 128).transpose(0, 2, 1))           # [NC, 128, WPC]

    w1x = np.ascontiguousarray(W1[0:F_X], dtype=_BF16)                 # [64,128]
    w1e = np.ascontiguousarray(W1[F_X:F_X + F_E], dtype=_BF16)         # [16,128]
    w1u = np.ascontiguousarray(u @ W1[F_X + F_E:], dtype=_BF16)        # [64,128]
    w2 = np.ascontiguousarray(W2, dtype=_BF16)                         # [128,64]
    b1c = np.ascontiguousarray(b1, np.float32)
    b2c = np.ascontiguousarray(b2, np.float32)
    identc = np.eye(128, dtype=np.float32)
    wbytes = [w1x, w1e, w1u, w2, b1c, b2c, identc]

    stage_caps = [tuple(int(c) for c in caps[k * WPS:(k + 1) * WPS])
                  for k in range(STAGES)]
    in_maps = [{} for _ in range(NC)]
    for k in range(STAGES):
        layout, nbytes = _blob_layout(stage_caps[k])
        e0 = int(offs[k * WPS])
        e1 = e0 + int(sum(stage_caps[k])) * 128 * F_E
        s0, s1 = k * SPS, (k + 1) * SPS
        for ci in range(NC):
            blob = np.zeros(nbytes, np.uint8)

            def put(name, arr):
                bts = np.ascontiguousarray(arr).view(np.uint8).ravel()
                blob[layout[name]:layout[name] + bts.size] = bts

            put("edges", A[ci, e0:e1])
            put("xt", xt[ci][:, s0:s1])
            put("oh", oh[ci][:, s0:s1])
            put("invc", ivt[ci][:, k * WPS:(k + 1) * WPS])
            for nm, arr in zip(("w1x", "w1e", "w1u", "w2", "b1", "b2", "ident"),
                               wbytes):
                put(nm, arr)
            in_maps[ci][f"blob{k}"] = blob

    plan = dict(stage_caps=tuple(stage_caps), slot_of_local=slot_of_local)
    return plan, in_maps


def _postprocess(stage_outs, plan):
    """stage_outs: list of STAGES arrays [NC*GPS, F_OUT, 512] f16."""
    slot_of_local = plan["slot_of_local"]
    out = np.empty((NC * NPC, F_OUT), np.float32)
    o = np.concatenate(
        [so.reshape(NC, GPS, F_OUT, B * 128) for so in stage_outs], axis=1)
    for ci in range(NC):
        o2 = o[ci].transpose(1, 0, 2).reshape(F_OUT, SLOTS)
        out[ci * NPC:(ci + 1) * NPC] = o2[:, slot_of_local[ci]].T
    return out


# ------------------------------------------------------------- device side
def _build_stage(caps_k):
    import concourse.bacc as bacc
    import concourse.mybir as mybir
    import concourse.tile as tile
    from contextlib import ExitStack

    f32 = mybir.dt.float32
    bf16 = mybir.dt.bfloat16
    f16 = mybir.dt.float16
    fp8 = mybir.dt.float8e3
    u8 = mybir.dt.uint8
    AF = mybir.ActivationFunctionType

    caps_k = list(caps_k)
    layout, nbytes = _blob_layout(caps_k)
    eoffs = [0] * WPS
    for w in range(1, WPS):
        eoffs[w] = eoffs[w - 1] + caps_k[w - 1] * 128 * F_E

    nc = bacc.Bacc("TRN2", target_bir_lowering=False)
    blob_d = nc.dram_tensor("blob", [nbytes], u8, kind="ExternalInput")
    out_d = nc.dram_tensor("outT", [GPS, F_OUT, B * 128], f16,
                           kind="ExternalOutput")

    def view(name, dt, p, q):
        o = layout[name]
        sz = p * q * np.dtype(mybir.dt.np(dt)).itemsize
        return blob_d[o:o + sz].bitcast(dt).rearrange("(p q) -> p q", p=p)

    with tile.TileContext(nc) as tc, ExitStack() as ctx:
        consts = ctx.enter_context(tc.tile_pool(name="consts", bufs=1))
        edge_pool = ctx.enter_context(tc.tile_pool(name="edges", bufs=4))
        gsn_pool = ctx.enter_context(tc.tile_pool(name="gsn", bufs=4))
        gsc_pool = ctx.enter_context(tc.tile_pool(name="gsc", bufs=2))
        ea_pool = ctx.enter_context(tc.tile_pool(name="ea", bufs=2))
        hid_pool = ctx.enter_context(tc.tile_pool(name="hid", bufs=2))
        out_pool = ctx.enter_context(tc.tile_pool(name="outs", bufs=3))
        pse_pool = ctx.enter_context(
            tc.tile_pool(name="pse", bufs=2, space="PSUM"))
        psh_pool = ctx.enter_context(
            tc.tile_pool(name="psh", bufs=2, space="PSUM"))
        pso_pool = ctx.enter_context(
            tc.tile_pool(name="pso", bufs=2, space="PSUM"))

        w1x_t = consts.tile([F_X, H], bf16)
        nc.sync.dma_start(w1x_t[:], view("w1x", bf16, F_X, H))
        w1e_t = consts.tile([F_E, H], bf16)
        nc.sync.dma_start(w1e_t[:], view("w1e", bf16, F_E, H))
        w1u_t = consts.tile([N_GRAPHS, H], bf16)
        nc.sync.dma_start(w1u_t[:], view("w1u", bf16, N_GRAPHS, H))
        w2_t = consts.tile([H, F_OUT], bf16)
        nc.sync.dma_start(w2_t[:], view("w2", bf16, H, F_OUT))
        b1_t = consts.tile([H, 1], f32)
        nc.sync.dma_start(b1_t[:], view("b1", f32, H, 1))
        b2_t = consts.tile([F_OUT, 1], f32)
        nc.sync.dma_start(b2_t[:], view("b2", f32, F_OUT, 1))
        invc_t = consts.tile([128, WPS], f32)
        nc.sync.dma_start(invc_t[:], view("invc", f32, 128, WPS))

        if XT_FP8:
            xt8_t = consts.tile([F_X, SPS], fp8)
            nc.sync.dma_start(xt8_t[:], view("xt", fp8, F_X, SPS))
            xt_t = consts.tile([F_X, SPS], bf16)
            nc.vector.tensor_copy(xt_t[:], xt8_t[:])
        else:
            xt_t = consts.tile([F_X, SPS], bf16)
            nc.sync.dma_start(xt_t[:], view("xt", bf16, F_X, SPS))
        oh_t = consts.tile([N_GRAPHS, SPS], fp8)
        nc.sync.dma_start(oh_t[:], view("oh", fp8, N_GRAPHS, SPS))
        ohb_t = consts.tile([N_GRAPHS, SPS], bf16)
        nc.vector.tensor_copy(ohb_t[:], oh_t[:])

        ident_t = consts.tile([128, 128], f32)
        nc.sync.dma_start(ident_t[:], view("ident", f32, 128, 128))

        eb = layout["edges"]
        for g in range(GPS):
            gsc = gsc_pool.tile([128, B * F_E], f32)
            for j in range(B):
                w = g * B + j
                cw = caps_k[w]
                et = edge_pool.tile([128, F_E * cw], fp8)
                src = blob_d[eb + eoffs[w]:eb + eoffs[w] + 128 * F_E * cw]
                nc.sync.dma_start(
                    et[:], src.bitcast(fp8).rearrange("(p q) -> p q", p=128))
                gsn = gsn_pool.tile([128, F_E], f32)
                nc.vector.tensor_reduce(
                    out=gsn[:],
                    in_=et[:].rearrange("p (f e) -> p f e", e=cw),
                    axis=mybir.AxisListType.X,
                    op=mybir.AluOpType.add,
                )
                nc.scalar.activation(
                    gsc[:, j * F_E:(j + 1) * F_E], gsn[:], AF.Identity,
                    scale=invc_t[:, w:w + 1],
                )

            pse = pse_pool.tile([F_E, B * 128], f32)
            for j in range(B):
                nc.tensor.transpose(
                    pse[:, j * 128:(j + 1) * 128],
                    gsc[:, j * F_E:(j + 1) * F_E],
                    ident_t[:],
                )
            ea = ea_pool.tile([F_E, B * 128], bf16)
            nc.vector.tensor_copy(ea[:], pse[:])

            psh = psh_pool.tile([H, B * 128], f32)
            nc.tensor.matmul(psh[:], w1e_t[:], ea[:], start=True, stop=False)
            nc.tensor.matmul(psh[:], w1x_t[:],
                             xt_t[:, g * 512:(g + 1) * 512],
                             start=False, stop=False)
            nc.tensor.matmul(psh[:], w1u_t[:],
                             ohb_t[:, g * 512:(g + 1) * 512],
                             start=False, stop=True)
            hid = hid_pool.tile([H, B * 128], bf16)
            nc.scalar.activation(hid[:], psh[:], AF.Relu, bias=b1_t[:])

            pso = pso_pool.tile([F_OUT, B * 128], f32)
            nc.tensor.matmul(pso[:], w2_t[:], hid[:], start=True, stop=True)
            outs = out_pool.tile([F_OUT, B * 128], f16)
            nc.scalar.activation(outs[:], pso[:], AF.Identity, bias=b2_t[:])
            nc.sync.dma_start(out_d[g], outs[:])

    nc.finalize()
    return nc


def _get_program(caps_k):
    key = (caps_k, XT_FP8)
    if key not in _PROGRAM_CACHE:
        _PROGRAM_CACHE[key] = _build_stage(caps_k)
    return _PROGRAM_CACHE[key]


# ------------------------------------------------------------- runner
class _Stage:
    def __init__(self, nc, sharding, mesh, jax, bass2jax, mybir):
        partition_name = (nc.partition_id_tensor.name
                          if nc.partition_id_tensor else None)
        in_names, out_names, out_avals = [], [], []
        for alloc in nc.m.functions[0].allocations:
            if not isinstance(alloc, mybir.MemoryLocationSet):
                continue
            name = alloc.memorylocations[0].name
            if alloc.kind == "ExternalInput":
                if name != partition_name:
                    in_names.append(name)
            elif alloc.kind == "ExternalOutput":
                out_names.append(name)
                out_avals.append(jax.core.ShapedArray(
                    tuple(alloc.tensor_shape), mybir.dt.np(alloc.dtype)))
        in_names_all = list(in_names) + out_names
        if partition_name is not None:
            in_names_all.append(partition_name)

        def _body(*args):
            operands = list(args)
            if partition_name is not None:
                operands.append(bass2jax.partition_id_tensor())
            return tuple(bass2jax._bass_exec_p.bind(
                *operands,
                out_avals=tuple(out_avals),
                in_names=tuple(in_names_all),
                out_names=tuple(out_names),
                lowering_input_output_aliases=(),
                sim_require_finite=True,
                sim_require_nnan=True,
                nc=nc,
            ))

        from jax.sharding import PartitionSpec
        from jax.experimental.shard_map import shard_map
        n_ops = len(in_names) + len(out_names)
        self.sharded = jax.jit(
            shard_map(_body, mesh=mesh,
                      in_specs=(PartitionSpec("core"),) * n_ops,
                      out_specs=(PartitionSpec("core"),) * len(out_names),
                      check_rep=False),
            keep_unused=True,
        )
        # output-named dummy operands: never read by the NEFF (the kernel
        # writes every output element), device-resident, reused every call
        self.dummy_outs = [
            jax.device_put(np.zeros((NC * a.shape[0], *a.shape[1:]), a.dtype),
                           sharding)
            for a in out_avals
        ]


class _Runner:
    def __init__(self, plan):
        import jax
        from jax.sharding import Mesh, PartitionSpec, NamedSharding
        import concourse.bass2jax as bass2jax
        import concourse.mybir as mybir

        self.jax = jax
        bass2jax.install_neuronx_cc_hook()
        devices = jax.devices()[:NC]
        mesh = Mesh(np.asarray(devices), ("core",))
        self.sharding = NamedSharding(mesh, PartitionSpec("core"))
        self.stages = [
            _Stage(_get_program(plan["stage_caps"][k]), self.sharding, mesh,
                   jax, bass2jax, mybir)
            for k in range(STAGES)
        ]
        self.pool = ThreadPoolExecutor(2)
        self.jax.block_until_ready(
            [d for s in self.stages for d in s.dummy_outs])

    def call(self, in_maps):
        """Preprocessed per-core blobs -> per-stage host output arrays."""
        futs = []
        for k in range(STAGES):
            g = np.concatenate([in_maps[ci][f"blob{k}"] for ci in range(NC)])
            d = self.jax.device_put(g, self.sharding)           # async H2D
            o = self.stages[k].sharded(d, *self.stages[k].dummy_outs)
            futs.append(self.pool.submit(np.asarray, o[0]))     # D2H thread
        return [f.result() for f in futs]


def _get_runner(plan):
    key = (plan["stage_caps"], XT_FP8)
    if key not in _RUNNER_CACHE:
        _RUNNER_CACHE[key] = _Runner(plan)
    return _RUNNER_CACHE[key]


def run(inputs):
    plan, in_maps = _plan_and_preprocess(inputs)
    runner = _get_runner(plan)
    stage_outs = runner.call(in_maps)
    return _postprocess(stage_outs, plan), plan, in_maps, runner


def kernel(**inputs):
    return run(inputs)[0]
